# revision 1
# baseline (speedup 1.0000x reference)
"""Trainium2 Bass kernel for nn_AgentPolicy (single-query attention policy net).

Reference computation (B=4096, N=64, FIN=256, D1=512, D2=128):
    x = obs_x @ W1 + b1                        [B, D1]
    y = others @ W1 + b1                       [B, N, D1]
    alpha = (x . y_n) / sqrt(D1)               [B, N]
    beta = softmax(alpha)                      [B, N]
    c = sum_n beta_n y_n                       [B, D1]
    out = concat([x, c])                       [B, 2*D1]
    out1 = softmax(out @ W2 + b2)              [B, D2]
    logits = out1 + NEG * (1 - mask)           [B, D2]
    (value head is dead code)

Algebraic reformulation used here (avoids materializing y: ~15x less flops):
    q = (x @ W1^T) / sqrt(D1)            [B, FIN]
    alpha_n = others_n . q  (+ const/b1 shift, cancelled by softmax)
    c = (beta^T others) @ W1 + b1  (sum beta = 1)
    out @ W2 = x @ W2a + s @ (W1 @ W2b) + b1 @ W2b,  s = beta^T others

Engine assignment (all fp32; hot matmuls run the PE's float32r
single-pass mode, 1 cyc/row at >=256 moving cols, so `others` never
needs an fp16 copy):
    DMA   : others/obs/mask loads (HWDGE), ~96us -- the HBM floor
    DVE   : alpha dot products (scalar_tensor_tensor w/ accum), logits
    ACT   : exps, PSUM->SBUF copies, bias adds, mask addend
    Pool  : diag(betau) builds (otherwise idle engine)
    PE    : x^T, q, weighted-sum (diag matmuls), out projections

Sharding: pure data-parallel over B across 8 cores (512 rows/core).
"""

import math

import numpy as np

import concourse.bass as bass
import concourse.mybir as mybir
import concourse.tile as tile
from concourse import bacc
from concourse.bass_utils import run_bass_kernel_spmd
from concourse.masks import make_identity

B, N, FIN, D1, D2 = 4096, 64, 256, 512, 128
NEG = -10000000.0
NCORES = 8
P = 128
KF = FIN // P          # 2 f-chunks of W1 contraction
KD = D1 // P           # 4 d-chunks
NCH = 8                # "others" n's per DMA chunk
NCHUNKS = N // NCH     # 8 chunks per row-tile
F32 = mybir.dt.float32
F32R = mybir.dt.float32r
I32 = mybir.dt.int32
AX = mybir.AxisListType
OP = mybir.AluOpType
AF = mybir.ActivationFunctionType



def build_nc(bc):
    """Build the per-core program. bc = batch rows handled by this core."""
    assert bc % P == 0
    rt = bc // P  # number of 128-row tiles
    nc = bacc.Bacc("TRN2")

    obs_d = nc.dram_tensor("obs_x", [bc, FIN], F32, kind="ExternalInput")
    oth_d = nc.dram_tensor("others", [bc, N, FIN], F32, kind="ExternalInput")
    am_d = nc.dram_tensor("action_mask", [bc, D2], I32, kind="ExternalInput")
    w1_d = nc.dram_tensor("W1", [FIN, D1], F32, kind="ExternalInput")
    b1_d = nc.dram_tensor("b1", [D1], F32, kind="ExternalInput")
    w2_d = nc.dram_tensor("W2", [2 * D1, D2], F32, kind="ExternalInput")
    b2_d = nc.dram_tensor("b2", [D2], F32, kind="ExternalInput")
    out_d = nc.dram_tensor("out", [bc, D2], F32, kind="ExternalOutput")

    with tile.TileContext(nc) as tc:
        with (
            tc.tile_pool(name="wpool", bufs=1) as wp,
            tc.tile_pool(name="sb", bufs=3) as sbp,
            tc.tile_pool(name="scr", bufs=3) as scrp,
            tc.tile_pool(name="oth", bufs=14) as othp,
            tc.tile_pool(name="dg", bufs=6) as dgp,
            tc.tile_pool(name="small", bufs=4) as smp,
            tc.tile_pool(name="psx", bufs=1, space="PSUM") as psx,
            tc.tile_pool(name="psq", bufs=1, space="PSUM") as psq,
            tc.tile_pool(name="pst_o", bufs=2, space="PSUM") as pst_o,
            tc.tile_pool(name="pst_s", bufs=1, space="PSUM") as pst_s,
            tc.tile_pool(name="pss", bufs=2, space="PSUM") as pss,
            tc.tile_pool(name="pso", bufs=1, space="PSUM") as pso,
        ):
            # ---------------- one-time setup ----------------
            ident = wp.tile([P, P], F32)
            make_identity(nc, ident[:])

            w1_sb = wp.tile([P, KF, D1], F32)       # W1[f, d], f-chunked
            for kf in range(KF):
                nc.sync.dma_start(w1_sb[:, kf, :], w1_d[kf * P:(kf + 1) * P, :])

            w2_sb = wp.tile([P, 2 * KD, D2], F32R)  # W2[d, d2], d-chunked

            b1_sb = wp.tile([P, KD], F32R)          # b1[d] as [128, KD]
            nc.sync.dma_start(
                b1_sb[:],
                b1_d.ap().rearrange("(k p) -> p k", p=P).bitcast(F32R))
            b2_sb = wp.tile([1, D2], F32)
            nc.sync.dma_start(b2_sb[:], b2_d.ap().rearrange("(a d) -> a d", a=1))

            ones_sb = wp.tile([1, P], F32)
            nc.vector.memset(ones_sb[:], 1.0)
            neg1_sb = wp.tile([P, 1], F32)
            nc.vector.memset(neg1_sb[:], NEG)

            # W1T[d, f] (d-chunked) via PE transposes
            w1t_sb = wp.tile([P, KD, FIN], F32R)
            for kd in range(KD):
                for kf in range(KF):
                    tp = pst_o.tile([P, P], F32, tag="pst_o")
                    nc.tensor.transpose(
                        tp[:], w1_sb[:, kf, kd * P:(kd + 1) * P], ident[:]
                    )
                    nc.scalar.copy(w1t_sb[:, kd, kf * P:(kf + 1) * P], tp[:])

            # W12[f, d2] = W1 @ W2b and cvec = b1 @ W2b + b2 -- emitted
            # after the first row-tile's prologue so the one-time setup
            # doesn't crowd the pipeline fill.
            w12_sb = wp.tile([P, KF, D2], F32R)
            cvec_sb = wp.tile([1, D2], F32)

            def build_w12_cvec():
                for j in range(2 * KD):
                    nc.sync.dma_start(w2_sb[:, j, :],
                                      w2_d[j * P:(j + 1) * P, :].bitcast(F32R))
                for kf in range(KF):
                    ps = pst_o.tile([P, P], F32, tag="pst_o")
                    for kd in range(KD):
                        nc.tensor.matmul(
                            ps[:, :D2],
                            w1t_sb[:, kd, kf * P:(kf + 1) * P],
                            w2_sb[:, KD + kd, :],
                            start=(kd == 0),
                            stop=(kd == KD - 1),
                        )
                    nc.scalar.copy(w12_sb[:, kf, :], ps[:, :D2])

                cps = pst_o.tile([P, P], F32, tag="pst_o")
                for kd in range(KD):
                    nc.tensor.matmul(
                        cps[:1, :D2],
                        b1_sb[:, kd:kd + 1],
                        w2_sb[:, KD + kd, :],
                        start=(kd == 0),
                        stop=(kd == KD - 1),
                    )
                nc.vector.tensor_add(cvec_sb[:], cps[:1, :D2], b2_sb[:])

            # ---------------- pipelined row tiles ----------------
            def prologue(t):
                """Loads + obs^T + xT + q for row-tile t (PE/ACT/DMA)."""
                r0 = t * P
                st = {}
                obs_t = sbp.tile([P, FIN], F32, tag="obs", name=f"obs{t}")
                nc.sync.dma_start(obs_t[:], obs_d[r0:r0 + P, :])
                mask_t = sbp.tile([P, D2], I32, tag="mask", name=f"mask{t}")
                nc.sync.dma_start(mask_t[:], am_d[r0:r0 + P, :])

                oth_t = []
                for c in range(NCHUNKS):
                    oc = othp.tile([P, NCH, FIN], F32R, tag="oth",
                                   name=f"oc{t}_{c}")
                    nc.sync.dma_start(
                        oc[:],
                        oth_d[r0:r0 + P, c * NCH:(c + 1) * NCH, :].bitcast(F32R))
                    oth_t.append(oc)
                st["oth"] = oth_t

                obsT = sbp.tile([P, KF, P], F32, tag="obsT", name=f"obsT{t}")
                for kf in range(KF):
                    tp = pst_o.tile([P, P], F32, tag="pst_o")
                    nc.tensor.transpose(
                        tp[:], obs_t[:, kf * P:(kf + 1) * P], ident[:]
                    )
                    nc.scalar.copy(obsT[:, kf, :], tp[:])

                # addend = NEG * (1 - mask), built on ACT:
                # maskf = float(mask); addend = Identity(-NEG*maskf + NEG)
                maskf = sbp.tile([P, D2], F32, tag="maskf", name=f"mf{t}")
                nc.scalar.copy(maskf[:], mask_t[:])
                addend = sbp.tile([P, D2], F32, tag="addend", name=f"ad{t}")
                nc.scalar.activation(
                    addend[:], maskf[:], AF.Identity,
                    bias=neg1_sb[:], scale=-NEG,
                )
                st["addend"] = addend

                xt_ps = psx.tile([P, KD, P], F32, tag="psx")
                for kd in range(KD):
                    for kf in range(KF):
                        nc.tensor.matmul(
                            xt_ps[:, kd, :],
                            w1_sb[:, kf, kd * P:(kd + 1) * P],
                            obsT[:, kf, :],
                            start=(kf == 0),
                            stop=(kf == KF - 1),
                        )
                xt_sb = sbp.tile([P, KD, P], F32R, tag="xt", name=f"xt{t}")
                for kd in range(KD):
                    nc.scalar.activation(
                        xt_sb[:, kd, :], xt_ps[:, kd, :], AF.Identity,
                        bias=b1_sb[:, kd:kd + 1].bitcast(F32), scale=1.0,
                    )
                st["xt"] = xt_sb

                q_ps = psq.tile([P, FIN], F32, tag="psq")
                for kd in range(KD):
                    nc.tensor.matmul(
                        q_ps[:],
                        xt_sb[:, kd, :],
                        w1t_sb[:, kd, :],
                        start=(kd == 0),
                        stop=(kd == KD - 1),
                    )
                q_sb = sbp.tile([P, FIN], F32, tag="q", name=f"q{t}")
                nc.scalar.mul(q_sb[:], q_ps[:], 1.0 / math.sqrt(float(D1)))
                st["q"] = q_sb
                return st

            def alpha_softmax(t, st):
                """Chunk-pipelined attention core: per 8-n chunk, alpha dot
                products (DVE), exp (ACT, no max subtraction -- values are
                bounded so fp32-safe; softmax is shift invariant), diag
                builds (Pool) and the weighted-sum matmuls (PE, float32r).
                The s normalization by 1/sum(exp) happens later on the PSUM
                read-out, so nothing here waits for the full softmax."""
                oth_t, q_sb = st["oth"], st["q"]
                alpha = sbp.tile([P, N], F32, tag="alpha", name=f"al{t}")
                betau = sbp.tile([P, N], F32, tag="betau", name=f"bu{t}")
                s_ps = pss.tile([P, FIN], F32, tag="pss")
                for c in range(NCHUNKS):
                    csl = slice(c * NCH, (c + 1) * NCH)
                    oc = oth_t[c]
                    for j in range(NCH):
                        n = c * NCH + j
                        scr = scrp.tile([P, FIN], F32, tag="scr")
                        nc.vector.scalar_tensor_tensor(
                            out=scr[:],
                            in0=oc[:, j, :].bitcast(F32),
                            scalar=1.0,
                            in1=q_sb[:],
                            op0=OP.mult,
                            op1=OP.mult,
                            accum_out=alpha[:, n:n + 1],
                        )
                    nc.scalar.activation(
                        betau[:, csl], alpha[:, csl], AF.Exp,
                        bias=0.0, scale=1.0,
                    )
                    # 8 diag matrices per chunk: DG[b, j, b'] =
                    # ident[b, b'] * betau[b, n]. Split DVE/ACT for engine
                    # balance (GpSimd is avoided: its SBUF port is shared
                    # with DVE and streaming on it stalls the alpha dots).
                    dgc = dgp.tile([P, NCH, P], F32R, tag="dg",
                                   name=f"dg{t}_{c}")
                    if c % 4 == 1:
                        nc.vector.tensor_tensor(
                            dgc[:],
                            ident[:].rearrange("p (o b) -> p o b", o=1)
                                    .broadcast_to([P, NCH, P]),
                            betau[:, csl].rearrange("p (n o) -> p n o", o=1)
                                         .broadcast_to([P, NCH, P]),
                            op=OP.mult,
                        )
                    else:
                        # reversed j: the chunk's 8 matmuls each wait on
                        # their own diag; writing diag j=0 LAST makes MM 0
                        # the gate, after which MMs 0..7 run back-to-back
                        # instead of dripping at ACT pace through the PE
                        # FIFO (which head-of-line-blocks later PE work).
                        for j in reversed(range(NCH)):
                            n = c * NCH + j
                            nc.scalar.mul(dgc[:, j, :], ident[:],
                                          betau[:, n:n + 1])
                    for j in range(NCH):
                        n = c * NCH + j
                        nc.tensor.matmul(
                            s_ps[:], dgc[:, j, :], oc[:, j, :],
                            start=(n == 0), stop=(n == N - 1),
                        )
                sumexp = smp.tile([P, 1], F32, tag="sumexp")
                nc.vector.reduce_sum(sumexp[:], betau[:], axis=AX.X)
                rbeta = smp.tile([P, 1], F32, tag="rbeta")
                nc.vector.reciprocal(rbeta[:], sumexp[:])
                st["s_ps"], st["rbeta"] = s_ps, rbeta

            def tail(t, st):
                """s normalization, sT, out_pre, softmax2+mask, store."""
                r0 = t * P
                s_ps, rbeta = st["s_ps"], st["rbeta"]
                xt_sb = st["xt"]

                s_sb = sbp.tile([P, FIN], F32, tag="s", name=f"s{t}")
                nc.scalar.mul(s_sb[:], s_ps[:], rbeta[:])

                sT = sbp.tile([P, KF, P], F32R, tag="sT", name=f"sT{t}")
                for kf in range(KF):
                    tp = pst_s.tile([P, P], F32, tag="pst_s")
                    nc.tensor.transpose(
                        tp[:], s_sb[:, kf * P:(kf + 1) * P], ident[:]
                    )
                    nc.scalar.copy(sT[:, kf, :], tp[:])

                o_ps = pso.tile([P, D2], F32, tag="pso")
                nc.tensor.matmul(
                    o_ps[:], ones_sb[:], cvec_sb[:], start=True, stop=False,
                )
                for kd in range(KD):
                    nc.tensor.matmul(
                        o_ps[:], xt_sb[:, kd, :], w2_sb[:, kd, :],
                        start=False, stop=False,
                    )
                for kf in range(KF):
                    nc.tensor.matmul(
                        o_ps[:], sT[:, kf, :], w12_sb[:, kf, :],
                        start=False, stop=(kf == KF - 1),
                    )

                # softmax2 with NO DVE ops (any DVE tail op head-of-line
                # blocks the next tile's dot products in the DVE FIFO):
                # out1 = exp(o - ln(sumexp(o))), all on ACT. The mask addend
                # is stored first and out1 added on top with an accumulating
                # SWDGE DMA, so the DVE never touches the tail.
                exp2 = sbp.tile([P, D2], F32, tag="exp2", name=f"e2{t}")
                sumexp2 = smp.tile([P, 1], F32, tag="sumexp2")
                nc.scalar.activation(
                    exp2[:], o_ps[:], AF.Exp,
                    bias=0.0, scale=1.0, accum_out=sumexp2[:],
                )
                lse = smp.tile([P, 1], F32, tag="lse")
                nc.scalar.activation(lse[:], sumexp2[:], AF.Ln,
                                     bias=0.0, scale=1.0)
                nlse = smp.tile([P, 1], F32, tag="nlse")
                nc.scalar.activation(nlse[:], lse[:], AF.Copy,
                                     bias=0.0, scale=-1.0)
                pre = sbp.tile([P, D2], F32, tag="pre", name=f"pre{t}")
                nc.scalar.activation(pre[:], o_ps[:], AF.Exp,
                                     bias=nlse[:], scale=1.0)

                addend = st["addend"]
                nc.sync.dma_start(out_d[r0:r0 + P, :], addend[:])
                nc.gpsimd.dma_start(out_d[r0:r0 + P, :], pre[:],
                                    accum_op=OP.add)

            # pipeline: prologues run TWO tiles ahead so q(t+1) is ready
            # the moment tile t's dots finish, and tail(t) is emitted AFTER
            # alpha(t+1) -- the DVE/ACT queues are strict FIFO, so an
            # eagerly-emitted tail head-of-line-blocks the next tile's dot
            # products while the tail's PE/ACT chain resolves. Deferring it
            # one tile gives the chain a full tile span to complete.
            # W2/W12/cvec setup is deferred -- only tail(0) needs it.
            states = {0: prologue(0)}
            states[1] = prologue(1)
            build_w12_cvec()
            for t in range(rt):
                alpha_softmax(t, states[t])
                if t + 2 < rt:
                    states[t + 2] = prologue(t + 2)
                if t >= 1:
                    tail(t - 1, states[t - 1])
                    del states[t - 1]
            tail(rt - 1, states[rt - 1])

    nc.finalize()
    return nc


_NC_CACHE = {}


def _get_nc(bc):
    if bc not in _NC_CACHE:
        _NC_CACHE[bc] = build_nc(bc)
    return _NC_CACHE[bc]


def kernel(obs_x, others, action_mask, W1, b1, W2, b2, W3, b3, W4, b4,
           trace=False):
    obs_x = np.ascontiguousarray(np.asarray(obs_x, dtype=np.float32))
    others = np.ascontiguousarray(np.asarray(others, dtype=np.float32))
    action_mask = np.ascontiguousarray(np.asarray(action_mask, dtype=np.int32))
    W1 = np.ascontiguousarray(np.asarray(W1, dtype=np.float32))
    b1 = np.ascontiguousarray(np.asarray(b1, dtype=np.float32))
    W2 = np.ascontiguousarray(np.asarray(W2, dtype=np.float32))
    b2 = np.ascontiguousarray(np.asarray(b2, dtype=np.float32))

    bc = B // NCORES
    nc = _get_nc(bc)
    in_maps = []
    for c in range(NCORES):
        sl = slice(c * bc, (c + 1) * bc)
        in_maps.append({
            "obs_x": obs_x[sl],
            "others": others[sl],
            "action_mask": action_mask[sl],
            "W1": W1, "b1": b1, "W2": W2, "b2": b2,
        })
    res = run_bass_kernel_spmd(nc, in_maps, list(range(NCORES)), trace=trace)
    out = np.concatenate([res.results[c]["out"] for c in range(NCORES)], axis=0)
    if trace:
        return out, res
    return out



# revision 3
# speedup vs baseline: 1.0089x; 1.0089x over previous
"""Trainium2 Bass kernel for nn_AgentPolicy (single-query attention policy net).

Reference computation (B=4096, N=64, FIN=256, D1=512, D2=128):
    x = obs_x @ W1 + b1                        [B, D1]
    y = others @ W1 + b1                       [B, N, D1]
    alpha = (x . y_n) / sqrt(D1)               [B, N]
    beta = softmax(alpha)                      [B, N]
    c = sum_n beta_n y_n                       [B, D1]
    out = concat([x, c])                       [B, 2*D1]
    out1 = softmax(out @ W2 + b2)              [B, D2]
    logits = out1 + NEG * (1 - mask)           [B, D2]
    (value head is dead code)

Algebraic reformulation (avoids materializing y: ~15x fewer flops):
    q = (x @ W1^T) / sqrt(D1)            [B, FIN]
    alpha_n = others_n . q  (+ const/b1 shift, cancelled by softmax)
    c = (beta^T others) @ W1 + b1  (sum beta = 1)
    out @ W2 = x @ W2a + s @ (W1 @ W2b) + b1 @ W2b,  s = beta^T others

The attention core runs in bf16 (validated: unmasked max abs err ~7e-4,
well inside the 2e-2 gate):
  - `others`/`obs_x` are cast fp32->bf16 *during* the DMA (SWDGE cast
    load, measured at full line rate) -- halves SBUF traffic/footprint.
  - All hot matmuls are bf16: 1 cyc/row (vs ~4 for fp32) + automatic
    FWL 4x weight loads.  The fp32 path measured 189ns LDW + 235ns MM
    per 128x128x256; bf16 cuts the PE diag-matmul chain ~3x.
  - The alpha dot products (DVE scalar_tensor_tensor w/ accum) run in
    the 2x_1p packed mode on bf16 operands.
  - PSUM accumulation stays fp32, softmax tail stays fp32.

Engine assignment:
    DMA   : SWDGE cast loads (others/obs/W1/W2), HWDGE mask/out
    DVE   : alpha dots, most diag(beta) builds, logits add
    ACT   : exps, PSUM->SBUF copies, bias adds, 2 diag chunks
    PE    : transposes, q/xt, diag-matmul weighted sum, out projections

Sharding: pure data-parallel over B across 8 cores (512 rows/core).
"""

import math

import numpy as np

import concourse.bass as bass
import concourse.mybir as mybir
import concourse.tile as tile
from concourse import bacc
from concourse.bass_utils import run_bass_kernel_spmd
from concourse.masks import make_identity

B, N, FIN, D1, D2 = 4096, 64, 256, 512, 128
NEG = -10000000.0
NCORES = 8
P = 128
KF = FIN // P          # 2 f-chunks of W1 contraction
KD = D1 // P           # 4 d-chunks
NCH = 8                # "others" n's per DMA chunk
NCHUNKS = N // NCH     # 8 chunks per row-tile
ACT_DIAG_CHUNKS = (1, 5)  # diag chunks built on ACT (rest on DVE)
F32 = mybir.dt.float32
BF16 = mybir.dt.bfloat16
I32 = mybir.dt.int32
AX = mybir.AxisListType
OP = mybir.AluOpType
AF = mybir.ActivationFunctionType


def build_nc(bc):
    """Build the per-core program. bc = batch rows handled by this core."""
    assert bc % P == 0
    rt = bc // P  # number of 128-row tiles
    nc = bacc.Bacc("TRN2")

    obs_d = nc.dram_tensor("obs_x", [bc, FIN], F32, kind="ExternalInput")
    oth_d = nc.dram_tensor("others", [bc, N, FIN], F32, kind="ExternalInput")
    am_d = nc.dram_tensor("action_mask", [bc, D2], I32, kind="ExternalInput")
    w1_d = nc.dram_tensor("W1", [FIN, D1], F32, kind="ExternalInput")
    b1_d = nc.dram_tensor("b1", [D1], F32, kind="ExternalInput")
    w2_d = nc.dram_tensor("W2", [2 * D1, D2], F32, kind="ExternalInput")
    b2_d = nc.dram_tensor("b2", [D2], F32, kind="ExternalInput")
    out_d = nc.dram_tensor("out", [bc, D2], F32, kind="ExternalOutput")

    with tile.TileContext(nc) as tc:
        with (
            tc.tile_pool(name="wpool", bufs=1) as wp,
            tc.tile_pool(name="sb", bufs=3) as sbp,
            tc.tile_pool(name="scr", bufs=3) as scrp,
            tc.tile_pool(name="oth", bufs=20) as othp,
            tc.tile_pool(name="dg", bufs=6) as dgp,
            tc.tile_pool(name="small", bufs=4) as smp,
            tc.tile_pool(name="psx", bufs=1, space="PSUM") as psx,
            tc.tile_pool(name="psq", bufs=1, space="PSUM") as psq,
            tc.tile_pool(name="pst_o", bufs=2, space="PSUM") as pst_o,
            tc.tile_pool(name="pst_s", bufs=1, space="PSUM") as pst_s,
            tc.tile_pool(name="pss", bufs=2, space="PSUM") as pss,
            tc.tile_pool(name="pso", bufs=1, space="PSUM") as pso,
        ):
            # ---------------- one-time setup ----------------
            # weight cast-loads go first in the SWDGE FIFO so they land
            # before the first others chunk (xt/q need them early).
            w1_sb = wp.tile([P, KF, D1], BF16)      # W1[f, d] bf16, f-chunked
            for kf in range(KF):
                nc.gpsimd.dma_start(w1_sb[:, kf, :], w1_d[kf * P:(kf + 1) * P, :])
            w2_sb = wp.tile([P, 2 * KD, D2], BF16)  # W2[d, d2] bf16, d-chunked
            for j in range(2 * KD):
                nc.gpsimd.dma_start(w2_sb[:, j, :], w2_d[j * P:(j + 1) * P, :])

            b1_sb = wp.tile([P, KD], F32)           # b1[d] as [128, KD] (ACT bias)
            nc.sync.dma_start(
                b1_sb[:], b1_d.ap().rearrange("(k p) -> p k", p=P))
            b1_bf = wp.tile([P, KD], BF16)          # bf16 copy for cvec matmul
            nc.gpsimd.dma_start(
                b1_bf[:], b1_d.ap().rearrange("(k p) -> p k", p=P))
            b2_sb = wp.tile([1, D2], F32)
            nc.sync.dma_start(b2_sb[:], b2_d.ap().rearrange("(a d) -> a d", a=1))

            ident = wp.tile([P, P], F32)
            make_identity(nc, ident[:])
            identb = wp.tile([P, P], BF16)
            nc.scalar.copy(identb[:], ident[:])

            ones_sb = wp.tile([1, P], F32)
            nc.vector.memset(ones_sb[:], 1.0)
            neg1_sb = wp.tile([P, 1], F32)
            nc.vector.memset(neg1_sb[:], NEG)

            # W1T[d, f] bf16 (d-chunked) via PE transposes
            w1t_sb = wp.tile([P, KD, FIN], BF16)
            for kd in range(KD):
                for kf in range(KF):
                    tp = pst_o.tile([P, P], BF16, tag="pst_o")
                    nc.tensor.transpose(
                        tp[:], w1_sb[:, kf, kd * P:(kd + 1) * P], identb[:]
                    )
                    nc.scalar.copy(w1t_sb[:, kd, kf * P:(kf + 1) * P], tp[:])

            # W12[f, d2] = W1 @ W2b and cvec = b1 @ W2b + b2 -- emitted
            # after the first row-tile's prologue so the one-time setup
            # doesn't crowd the pipeline fill.
            w12_sb = wp.tile([P, KF, D2], BF16)
            cvec_sb = wp.tile([1, D2], F32)

            def build_w12_cvec():
                for kf in range(KF):
                    ps = pst_o.tile([P, P], F32, tag="pst_o")
                    for kd in range(KD):
                        nc.tensor.matmul(
                            ps[:, :D2],
                            w1t_sb[:, kd, kf * P:(kf + 1) * P],
                            w2_sb[:, KD + kd, :],
                            start=(kd == 0),
                            stop=(kd == KD - 1),
                        )
                    nc.scalar.copy(w12_sb[:, kf, :], ps[:, :D2])

                cps = pst_o.tile([P, P], F32, tag="pst_o")
                for kd in range(KD):
                    nc.tensor.matmul(
                        cps[:1, :D2],
                        b1_bf[:, kd:kd + 1],
                        w2_sb[:, KD + kd, :],
                        start=(kd == 0),
                        stop=(kd == KD - 1),
                    )
                nc.vector.tensor_add(cvec_sb[:], cps[:1, :D2], b2_sb[:])

            # ---------------- pipelined row tiles ----------------
            def prologue(t):
                """Loads + obs^T + xT + q for row-tile t (PE/ACT/DMA)."""
                r0 = t * P
                st = {}
                obs_t = sbp.tile([P, FIN], BF16, tag="obs", name=f"obs{t}")
                nc.gpsimd.dma_start(obs_t[:], obs_d[r0:r0 + P, :])
                mask_t = sbp.tile([P, D2], I32, tag="mask", name=f"mask{t}")
                nc.sync.dma_start(mask_t[:], am_d[r0:r0 + P, :])

                oth_t = []
                for c in range(NCHUNKS):
                    oc = othp.tile([P, NCH, FIN], BF16, tag="oth",
                                   name=f"oc{t}_{c}")
                    nc.gpsimd.dma_start(
                        oc[:], oth_d[r0:r0 + P, c * NCH:(c + 1) * NCH, :])
                    oth_t.append(oc)
                st["oth"] = oth_t

                obsT = sbp.tile([P, KF, P], BF16, tag="obsT", name=f"obsT{t}")
                for kf in range(KF):
                    tp = pst_o.tile([P, P], BF16, tag="pst_o")
                    nc.tensor.transpose(
                        tp[:], obs_t[:, kf * P:(kf + 1) * P], identb[:]
                    )
                    nc.scalar.copy(obsT[:, kf, :], tp[:])

                # addend = NEG * (1 - mask), built on ACT:
                # maskf = float(mask); addend = Identity(-NEG*maskf + NEG)
                maskf = sbp.tile([P, D2], F32, tag="maskf", name=f"mf{t}")
                nc.scalar.copy(maskf[:], mask_t[:])
                addend = sbp.tile([P, D2], F32, tag="addend", name=f"ad{t}")
                nc.scalar.activation(
                    addend[:], maskf[:], AF.Identity,
                    bias=neg1_sb[:], scale=-NEG,
                )
                st["addend"] = addend

                xt_ps = psx.tile([P, KD, P], F32, tag="psx")
                for kd in range(KD):
                    for kf in range(KF):
                        nc.tensor.matmul(
                            xt_ps[:, kd, :],
                            w1_sb[:, kf, kd * P:(kd + 1) * P],
                            obsT[:, kf, :],
                            start=(kf == 0),
                            stop=(kf == KF - 1),
                        )
                xt_sb = sbp.tile([P, KD, P], BF16, tag="xt", name=f"xt{t}")
                for kd in range(KD):
                    nc.scalar.activation(
                        xt_sb[:, kd, :], xt_ps[:, kd, :], AF.Identity,
                        bias=b1_sb[:, kd:kd + 1], scale=1.0,
                    )
                st["xt"] = xt_sb

                q_ps = psq.tile([P, FIN], F32, tag="psq")
                for kd in range(KD):
                    nc.tensor.matmul(
                        q_ps[:],
                        xt_sb[:, kd, :],
                        w1t_sb[:, kd, :],
                        start=(kd == 0),
                        stop=(kd == KD - 1),
                    )
                q_sb = sbp.tile([P, FIN], BF16, tag="q", name=f"q{t}")
                nc.scalar.mul(q_sb[:], q_ps[:], 1.0 / math.sqrt(float(D1)))
                st["q"] = q_sb
                return st

            def diag_build(t, c, betau):
                """dgc[b, j, b'] = ident[b, b'] * betau[b, c*8+j], bf16."""
                csl = slice(c * NCH, (c + 1) * NCH)
                dgc = dgp.tile([P, NCH, P], BF16, tag="dg", name=f"dg{t}_{c}")
                if c in ACT_DIAG_CHUNKS:
                    # reversed j: the chunk's 8 matmuls each wait on their
                    # own diag; writing diag j=0 LAST makes MM 0 the gate,
                    # after which MMs 0..7 run back-to-back.
                    for j in reversed(range(NCH)):
                        n = c * NCH + j
                        nc.scalar.mul(dgc[:, j, :], identb[:],
                                      betau[:, n:n + 1])
                else:
                    # one DVE op for the whole chunk (broadcast trick)
                    nc.vector.tensor_tensor(
                        dgc[:],
                        identb[:].rearrange("p (o b) -> p o b", o=1)
                                 .broadcast_to([P, NCH, P]),
                        betau[:, csl].rearrange("p (n o) -> p n o", o=1)
                                     .broadcast_to([P, NCH, P]),
                        op=OP.mult,
                    )
                return dgc

            def alpha_softmax(t, st):
                """Chunk-pipelined attention core: per 8-n chunk, alpha dot
                products (DVE, bf16 2x mode), exp (ACT, no max subtraction
                -- alpha in [-11, 11] so fp32/bf16-safe; softmax is shift
                invariant), diag builds (DVE bulk / ACT for 2 chunks) and
                the weighted-sum matmuls (PE, bf16).  DVE diag ops are
                emitted one chunk late so they never head-of-line-block the
                next chunk's dots in the strict DVE FIFO while waiting on
                exp.  The s normalization by 1/sum(exp) happens later on
                the PSUM read-out, so nothing here waits for the full
                softmax."""
                oth_t, q_sb = st["oth"], st["q"]
                alpha = sbp.tile([P, N], F32, tag="alpha", name=f"al{t}")
                betau = sbp.tile([P, N], F32, tag="betau", name=f"bu{t}")
                s_ps = pss.tile([P, FIN], F32, tag="pss")
                pending = []  # chunks whose diag+matmuls are not yet emitted

                def flush_chunk(c):
                    dgc = diag_build(t, c, betau)
                    oc = oth_t[c]
                    for j in range(NCH):
                        n = c * NCH + j
                        nc.tensor.matmul(
                            s_ps[:], dgc[:, j, :], oc[:, j, :],
                            start=(n == 0), stop=(n == N - 1),
                        )

                for c in range(NCHUNKS):
                    csl = slice(c * NCH, (c + 1) * NCH)
                    oc = oth_t[c]
                    for j in range(NCH):
                        n = c * NCH + j
                        scr = scrp.tile([P, FIN], BF16, tag="scr")
                        nc.vector.scalar_tensor_tensor(
                            out=scr[:],
                            in0=oc[:, j, :],
                            scalar=1.0,
                            in1=q_sb[:],
                            op0=OP.mult,
                            op1=OP.mult,
                            accum_out=alpha[:, n:n + 1],
                        )
                    nc.scalar.activation(
                        betau[:, csl], alpha[:, csl], AF.Exp,
                        bias=0.0, scale=1.0,
                    )
                    pending.append(c)
                    if len(pending) > 1:
                        flush_chunk(pending.pop(0))
                while pending:
                    flush_chunk(pending.pop(0))

                sumexp = smp.tile([P, 1], F32, tag="sumexp")
                nc.vector.reduce_sum(sumexp[:], betau[:], axis=AX.X)
                rbeta = smp.tile([P, 1], F32, tag="rbeta")
                nc.vector.reciprocal(rbeta[:], sumexp[:])
                st["s_ps"], st["rbeta"] = s_ps, rbeta

            def tail(t, st):
                """s normalization, sT, out_pre, softmax2+mask, store."""
                r0 = t * P
                s_ps, rbeta = st["s_ps"], st["rbeta"]
                xt_sb = st["xt"]

                s_sb = sbp.tile([P, FIN], BF16, tag="s", name=f"s{t}")
                nc.scalar.mul(s_sb[:], s_ps[:], rbeta[:])

                sT = sbp.tile([P, KF, P], BF16, tag="sT", name=f"sT{t}")
                for kf in range(KF):
                    tp = pst_s.tile([P, P], BF16, tag="pst_s")
                    nc.tensor.transpose(
                        tp[:], s_sb[:, kf * P:(kf + 1) * P], identb[:]
                    )
                    nc.scalar.copy(sT[:, kf, :], tp[:])

                o_ps = pso.tile([P, D2], F32, tag="pso")
                nc.tensor.matmul(
                    o_ps[:], ones_sb[:], cvec_sb[:], start=True, stop=False,
                )
                for kd in range(KD):
                    nc.tensor.matmul(
                        o_ps[:], xt_sb[:, kd, :], w2_sb[:, kd, :],
                        start=False, stop=False,
                    )
                for kf in range(KF):
                    nc.tensor.matmul(
                        o_ps[:], sT[:, kf, :], w12_sb[:, kf, :],
                        start=False, stop=(kf == KF - 1),
                    )

                # softmax2 on ACT: out1 = exp(o - ln(sumexp(o))).  The mask
                # addend is added on DVE (tiny op, emitted a tile late so it
                # never blocks dots) and stored via HWDGE.
                sumexp2 = smp.tile([P, 1], F32, tag="sumexp2")
                exp2 = sbp.tile([P, D2], F32, tag="exp2", name=f"e2{t}")
                nc.scalar.activation(
                    exp2[:], o_ps[:], AF.Exp,
                    bias=0.0, scale=1.0, accum_out=sumexp2[:],
                )
                lse = smp.tile([P, 1], F32, tag="lse")
                nc.scalar.activation(lse[:], sumexp2[:], AF.Ln,
                                     bias=0.0, scale=1.0)
                nlse = smp.tile([P, 1], F32, tag="nlse")
                nc.scalar.activation(nlse[:], lse[:], AF.Copy,
                                     bias=0.0, scale=-1.0)
                pre = sbp.tile([P, D2], F32, tag="pre", name=f"pre{t}")
                nc.scalar.activation(pre[:], o_ps[:], AF.Exp,
                                     bias=nlse[:], scale=1.0)

                logits = sbp.tile([P, D2], F32, tag="logits", name=f"lg{t}")
                nc.vector.tensor_add(logits[:], pre[:], st["addend"][:])
                nc.sync.dma_start(out_d[r0:r0 + P, :], logits[:])

            # pipeline: prologues run TWO tiles ahead so q(t+1) is ready
            # the moment tile t's dots finish, and tail(t) is emitted AFTER
            # alpha(t+1) -- the DVE/ACT queues are strict FIFO, so an
            # eagerly-emitted tail head-of-line-blocks the next tile's dot
            # products while the tail's PE/ACT chain resolves.  Deferring it
            # one tile gives the chain a full tile span to complete.
            # W12/cvec setup is deferred -- only tail(0) needs it.
            states = {0: prologue(0)}
            states[1] = prologue(1)
            build_w12_cvec()
            for t in range(rt):
                alpha_softmax(t, states[t])
                if t + 2 < rt:
                    states[t + 2] = prologue(t + 2)
                if t >= 1:
                    tail(t - 1, states[t - 1])
                    del states[t - 1]
            tail(rt - 1, states[rt - 1])

    nc.finalize()
    return nc


_NC_CACHE = {}


def _get_nc(bc):
    if bc not in _NC_CACHE:
        _NC_CACHE[bc] = build_nc(bc)
    return _NC_CACHE[bc]


def kernel(obs_x, others, action_mask, W1, b1, W2, b2, W3, b3, W4, b4,
           trace=False):
    obs_x = np.ascontiguousarray(np.asarray(obs_x, dtype=np.float32))
    others = np.ascontiguousarray(np.asarray(others, dtype=np.float32))
    action_mask = np.ascontiguousarray(np.asarray(action_mask, dtype=np.int32))
    W1 = np.ascontiguousarray(np.asarray(W1, dtype=np.float32))
    b1 = np.ascontiguousarray(np.asarray(b1, dtype=np.float32))
    W2 = np.ascontiguousarray(np.asarray(W2, dtype=np.float32))
    b2 = np.ascontiguousarray(np.asarray(b2, dtype=np.float32))

    bc = B // NCORES
    nc = _get_nc(bc)
    in_maps = []
    for c in range(NCORES):
        sl = slice(c * bc, (c + 1) * bc)
        in_maps.append({
            "obs_x": obs_x[sl],
            "others": others[sl],
            "action_mask": action_mask[sl],
            "W1": W1, "b1": b1, "W2": W2, "b2": b2,
        })
    res = run_bass_kernel_spmd(nc, in_maps, list(range(NCORES)), trace=trace)
    out = np.concatenate([res.results[c]["out"] for c in range(NCORES)], axis=0)
    if trace:
        return out, res
    return out


# revision 8
# speedup vs baseline: 1.1987x; 1.1882x over previous
"""Trainium2 Bass kernel for nn_AgentPolicy (single-query attention policy net).

Reference computation (B=4096, N=64, FIN=256, D1=512, D2=128):
    x = obs_x @ W1 + b1                        [B, D1]
    y = others @ W1 + b1                       [B, N, D1]
    alpha = (x . y_n) / sqrt(D1)               [B, N]
    beta = softmax(alpha)                      [B, N]
    c = sum_n beta_n y_n                       [B, D1]
    out = concat([x, c])                       [B, 2*D1]
    out1 = softmax(out @ W2 + b2)              [B, D2]
    logits = out1 + NEG * (1 - mask)           [B, D2]
    (value head is dead code)

Algebraic reformulation (avoids materializing y: ~15x fewer flops):
    q = (x @ W1^T) / sqrt(D1)            [B, FIN]
    alpha_n = others_n . q  (+ const/b1 shift, cancelled by softmax)
    c = (beta^T others) @ W1 + b1  (sum beta = 1)
    out @ W2 = x @ W2a + s @ (W1 @ W2b) + b1 @ W2b,  s = beta^T others

The attention core runs in bf16 (validated: unmasked max abs err ~8e-4,
well inside the 2e-2 gate):
  - `others`/`obs_x`/weights are cast fp32->bf16 *during* the DMA (SWDGE
    cast load, measured at full line rate) -- halves SBUF traffic.
  - All hot matmuls are bf16 (1 cyc/row + FWL weight loads).
  - The alpha dot products (DVE scalar_tensor_tensor w/ accum) have no
    DVE fast mode (measured: STT supports none; tensor_tensor only 2x),
    so DVE is budgeted at 1 elem/cyc for them: ~21.6us/tile -- which
    sets the pace together with the ~24us/tile HBM stream.
  - diag(beta) builds are spread across three engines so no single one
    exceeds the DMA pace: gpsimd local_scatter (zeros + writes just the
    diagonal), ACT per-n scalar muls, and DVE broadcast multiplies.
  - PSUM accumulation stays fp32; softmax tail stays fp32.

Sharding: pure data-parallel over B across 8 cores (512 rows/core).
"""

import math

import numpy as np

import concourse.bass as bass
import concourse.mybir as mybir
import concourse.tile as tile
from concourse import bacc
from concourse.bass_utils import run_bass_kernel_spmd
from concourse.masks import make_identity

B, N, FIN, D1, D2 = 4096, 64, 256, 512, 128
NEG = -10000000.0
NCORES = 8
P = 128
KF = FIN // P          # 2 f-chunks of W1 contraction
KD = D1 // P           # 4 d-chunks
NCH = 8                # "others" n's per compute chunk
NCHUNKS = N // NCH     # 8 chunks per row-tile
GRP = 2                # compute chunks per DMA group (tiles >= 1)
NGRP = NCHUNKS // GRP
# diag(beta) build engine per chunk: gpsimd scatter / ACT muls / DVE bcast
DIAG_ENGINE = {0: "gp", 1: "act", 2: "dve", 3: "gp",
               4: "act", 5: "dve", 6: "gp", 7: "act"}
F32 = mybir.dt.float32
BF16 = mybir.dt.bfloat16
I16 = mybir.dt.int16
I32 = mybir.dt.int32
AX = mybir.AxisListType
OP = mybir.AluOpType
AF = mybir.ActivationFunctionType


def build_nc(bc):
    """Build the per-core program. bc = batch rows handled by this core."""
    assert bc % P == 0
    rt = bc // P  # number of 128-row tiles
    nc = bacc.Bacc("TRN2")

    obs_d = nc.dram_tensor("obs_x", [bc, FIN], F32, kind="ExternalInput")
    oth_d = nc.dram_tensor("others", [bc, N, FIN], F32, kind="ExternalInput")
    am_d = nc.dram_tensor("action_mask", [bc, D2], I32, kind="ExternalInput")
    w1_d = nc.dram_tensor("W1", [FIN, D1], F32, kind="ExternalInput")
    b1_d = nc.dram_tensor("b1", [D1], F32, kind="ExternalInput")
    w2_d = nc.dram_tensor("W2", [2 * D1, D2], F32, kind="ExternalInput")
    b2_d = nc.dram_tensor("b2", [D2], F32, kind="ExternalInput")
    out_d = nc.dram_tensor("out", [bc, D2], F32, kind="ExternalOutput")

    with tile.TileContext(nc) as tc:
        with (
            tc.tile_pool(name="wpool", bufs=1) as wp,
            tc.tile_pool(name="sb", bufs=3) as sbp,
            tc.tile_pool(name="scr", bufs=3) as scrp,
            tc.tile_pool(name="oth", bufs=3 * NGRP) as othp,
            tc.tile_pool(name="dg", bufs=6) as dgp,
            tc.tile_pool(name="small", bufs=4) as smp,
            tc.tile_pool(name="psx", bufs=1, space="PSUM") as psx,
            tc.tile_pool(name="psq", bufs=1, space="PSUM") as psq,
            tc.tile_pool(name="pst_o", bufs=2, space="PSUM") as pst_o,
            tc.tile_pool(name="pst_s", bufs=1, space="PSUM") as pst_s,
            tc.tile_pool(name="pss", bufs=2, space="PSUM") as pss,
            tc.tile_pool(name="pso", bufs=1, space="PSUM") as pso,
        ):
            # ---------------- one-time setup ----------------
            # weight cast-loads go first in the SWDGE FIFO (single DMAs so
            # the prefix before the first others chunk stays ~3us).
            w1_sb = wp.tile([P, KF, D1], BF16)      # W1[f, d] bf16, f-chunked
            nc.gpsimd.dma_start(
                w1_sb[:], w1_d.ap().rearrange("(k p) d -> p k d", p=P))
            w2_sb = wp.tile([P, 2 * KD, D2], BF16)  # W2[d, d2] bf16, d-chunked
            nc.gpsimd.dma_start(
                w2_sb[:], w2_d.ap().rearrange("(j p) d -> p j d", p=P))
            b1_bf = wp.tile([P, KD], BF16)          # bf16 b1 for cvec matmul
            nc.gpsimd.dma_start(
                b1_bf[:], b1_d.ap().rearrange("(k p) -> p k", p=P))

            b1_sb = wp.tile([P, KD], F32)           # b1[d] as [128, KD] (ACT bias)
            nc.sync.dma_start(
                b1_sb[:], b1_d.ap().rearrange("(k p) -> p k", p=P))
            b2_sb = wp.tile([1, D2], F32)
            nc.sync.dma_start(b2_sb[:], b2_d.ap().rearrange("(a d) -> a d", a=1))

            ident = wp.tile([P, P], F32)
            make_identity(nc, ident[:])
            identb = wp.tile([P, P], BF16)
            nc.scalar.copy(identb[:], ident[:])
            # 0 on the diagonal, -30000 off it: Exp(identNEG + alpha_n)
            # yields diag(exp(alpha_n)) directly (off-diag underflows to 0)
            neg30k = wp.tile([P, 1], F32)
            nc.vector.memset(neg30k[:], -30000.0)
            identneg = wp.tile([P, P], BF16)
            nc.scalar.activation(identneg[:], ident[:], AF.Identity,
                                 bias=neg30k[:], scale=30000.0)

            # idx[p, j] = j*128 + p for the diag local_scatter
            dgidx = wp.tile([P, NCH], I16)
            nc.gpsimd.iota(dgidx[:], pattern=[[P, NCH]], base=0,
                           channel_multiplier=1)

            ones_sb = wp.tile([1, P], F32)
            nc.vector.memset(ones_sb[:], 1.0)

            # W1T[d, f] bf16 (d-chunked) via PE transposes
            w1t_sb = wp.tile([P, KD, FIN], BF16)
            for kd in range(KD):
                for kf in range(KF):
                    tp = pst_o.tile([P, P], BF16, tag="pst_o")
                    nc.tensor.transpose(
                        tp[:], w1_sb[:, kf, kd * P:(kd + 1) * P], identb[:]
                    )
                    nc.scalar.copy(w1t_sb[:, kd, kf * P:(kf + 1) * P], tp[:])

            # W12[f, d2] = W1 @ W2b and cvec = b1 @ W2b + b2 -- emitted
            # after alpha(0) so neither its PE chain nor its DVE add can
            # head-of-line-block the first tile's dot products (only
            # tail(0) needs the results).
            w12_sb = wp.tile([P, KF, D2], BF16)
            cvec_sb = wp.tile([1, D2], F32)

            def build_w12_cvec():
                for kf in range(KF):
                    ps = pst_o.tile([P, P], F32, tag="pst_o")
                    for kd in range(KD):
                        nc.tensor.matmul(
                            ps[:, :D2],
                            w1t_sb[:, kd, kf * P:(kf + 1) * P],
                            w2_sb[:, KD + kd, :],
                            start=(kd == 0),
                            stop=(kd == KD - 1),
                        )
                    nc.scalar.copy(w12_sb[:, kf, :], ps[:, :D2])

                cps = pst_o.tile([P, P], F32, tag="pst_o")
                for kd in range(KD):
                    nc.tensor.matmul(
                        cps[:1, :D2],
                        b1_bf[:, kd:kd + 1],
                        w2_sb[:, KD + kd, :],
                        start=(kd == 0),
                        stop=(kd == KD - 1),
                    )
                nc.vector.tensor_add(cvec_sb[:], cps[:1, :D2], b2_sb[:])

            # ---------------- pipelined row tiles ----------------
            def prologue(t):
                """Loads + obs^T + xT + q for row-tile t (PE/ACT/DMA).
                others are cast-loaded in groups; tile 0 uses single-chunk
                DMAs so the first dots start ~3us after the weight prefix."""
                r0 = t * P
                st = {}
                obs_t = sbp.tile([P, FIN], BF16, tag="obs", name=f"obs{t}")
                nc.gpsimd.dma_start(obs_t[:], obs_d[r0:r0 + P, :])
                mask_t = sbp.tile([P, D2], I32, tag="mask", name=f"mask{t}")
                nc.sync.dma_start(mask_t[:], am_d[r0:r0 + P, :])
                st["mask"] = mask_t

                chunks = []
                if t == 0:
                    for c in range(NCHUNKS):
                        oc = othp.tile([P, NCH, FIN], BF16, tag="oth0",
                                       name=f"oc{t}_{c}")
                        nc.gpsimd.dma_start(
                            oc[:], oth_d[r0:r0 + P, c * NCH:(c + 1) * NCH, :])
                        chunks.append(oc[:])
                else:
                    for g in range(NGRP):
                        og = othp.tile([P, GRP * NCH, FIN], BF16, tag="oth",
                                       name=f"og{t}_{g}")
                        nc.gpsimd.dma_start(
                            og[:],
                            oth_d[r0:r0 + P,
                                  g * GRP * NCH:(g + 1) * GRP * NCH, :])
                        for u in range(GRP):
                            chunks.append(og[:, u * NCH:(u + 1) * NCH, :])
                st["oth"] = chunks

                obsT = sbp.tile([P, KF, P], BF16, tag="obsT", name=f"obsT{t}")
                for kf in range(KF):
                    tp = pst_o.tile([P, P], BF16, tag="pst_o")
                    nc.tensor.transpose(
                        tp[:], obs_t[:, kf * P:(kf + 1) * P], identb[:]
                    )
                    nc.scalar.copy(obsT[:, kf, :], tp[:])

                xt_ps = psx.tile([P, KD, P], F32, tag="psx")
                for kd in range(KD):
                    for kf in range(KF):
                        nc.tensor.matmul(
                            xt_ps[:, kd, :],
                            w1_sb[:, kf, kd * P:(kd + 1) * P],
                            obsT[:, kf, :],
                            start=(kf == 0),
                            stop=(kf == KF - 1),
                        )
                xt_sb = sbp.tile([P, KD, P], BF16, tag="xt", name=f"xt{t}")
                for kd in range(KD):
                    nc.scalar.activation(
                        xt_sb[:, kd, :], xt_ps[:, kd, :], AF.Identity,
                        bias=b1_sb[:, kd:kd + 1], scale=1.0,
                    )
                st["xt"] = xt_sb

                q_ps = psq.tile([P, FIN], F32, tag="psq")
                for kd in range(KD):
                    nc.tensor.matmul(
                        q_ps[:],
                        xt_sb[:, kd, :],
                        w1t_sb[:, kd, :],
                        start=(kd == 0),
                        stop=(kd == KD - 1),
                    )
                q_sb = sbp.tile([P, FIN], BF16, tag="q", name=f"q{t}")
                nc.scalar.mul(q_sb[:], q_ps[:], 1.0 / math.sqrt(float(D1)))
                st["q"] = q_sb
                return st

            def diag_build(t, c, alpha, betau):
                """dgc[b, j, b'] = ident[b, b'] * exp(alpha[b, c*8+j]), bf16."""
                csl = slice(c * NCH, (c + 1) * NCH)
                dgc = dgp.tile([P, NCH, P], BF16, tag="dg", name=f"dg{t}_{c}")
                eng = DIAG_ENGINE[c]
                if eng == "gp":
                    # zeros the tile and writes just the 128 diagonal values
                    nc.gpsimd.local_scatter(
                        dgc[:], betau[:, csl], dgidx[:],
                        channels=P, num_elems=NCH * P, num_idxs=NCH,
                    )
                elif eng == "act":
                    # diag(exp(alpha_n)) straight from fp32 alpha (ACT scale
                    # APs must be fp32, so no betau multiply here).
                    # reversed j: the chunk's 8 matmuls each wait on their
                    # own diag; writing diag j=0 LAST makes MM 0 the gate,
                    # after which MMs 0..7 run back-to-back.
                    for j in reversed(range(NCH)):
                        n = c * NCH + j
                        nc.scalar.activation(
                            dgc[:, j, :], identneg[:], AF.Exp,
                            bias=alpha[:, n:n + 1], scale=1.0,
                        )
                else:
                    # one DVE op for the whole chunk (broadcast trick)
                    nc.vector.tensor_tensor(
                        dgc[:],
                        identb[:].rearrange("p (o b) -> p o b", o=1)
                                 .broadcast_to([P, NCH, P]),
                        betau[:, csl].rearrange("p (n o) -> p n o", o=1)
                                     .broadcast_to([P, NCH, P]),
                        op=OP.mult,
                    )
                return dgc

            def alpha_softmax(t, st):
                """Chunk-pipelined attention core: per 8-n chunk, alpha dot
                products (DVE), exp (ACT, no max subtraction -- alpha is in
                [-11, 11] so fp32-safe; softmax is shift invariant), diag
                builds (gp/act/dve per DIAG_ENGINE) and the weighted-sum
                matmuls (PE, bf16).  DVE-built diags are emitted one chunk
                late so they never head-of-line-block the next chunk's dots
                in the strict DVE FIFO while waiting on exp.  The s
                normalization by 1/sum(exp) happens later on the PSUM
                read-out, so nothing here waits for the full softmax."""
                oth_c, q_sb = st["oth"], st["q"]
                alpha = sbp.tile([P, N], F32, tag="alpha", name=f"al{t}")
                betau = sbp.tile([P, N], BF16, tag="betau", name=f"bu{t}")
                s_ps = pss.tile([P, FIN], F32, tag="pss")
                nmm = [0]
                pending = []

                def flush_chunk(c):
                    dgc = diag_build(t, c, alpha, betau)
                    oc = oth_c[c]
                    for j in range(NCH):
                        nc.tensor.matmul(
                            s_ps[:], dgc[:, j, :], oc[:, j, :],
                            start=(nmm[0] == 0), stop=(nmm[0] == N - 1),
                        )
                        nmm[0] += 1

                for c in range(NCHUNKS):
                    csl = slice(c * NCH, (c + 1) * NCH)
                    oc = oth_c[c]
                    for j in range(NCH):
                        n = c * NCH + j
                        scr = scrp.tile([P, FIN], BF16, tag="scr")
                        nc.vector.scalar_tensor_tensor(
                            out=scr[:],
                            in0=oc[:, j, :],
                            scalar=1.0,
                            in1=q_sb[:],
                            op0=OP.mult,
                            op1=OP.mult,
                            accum_out=alpha[:, n:n + 1],
                        )
                    nc.scalar.activation(
                        betau[:, csl], alpha[:, csl], AF.Exp,
                        bias=0.0, scale=1.0,
                    )
                    if DIAG_ENGINE[c] == "dve":
                        pending.append(c)
                        if len(pending) > 1:
                            flush_chunk(pending.pop(0))
                    else:
                        flush_chunk(c)
                while pending:
                    flush_chunk(pending.pop(0))

                sumexp = smp.tile([P, 1], F32, tag="sumexp")
                nc.vector.reduce_sum(sumexp[:], betau[:], axis=AX.X)
                rbeta = smp.tile([P, 1], F32, tag="rbeta")
                nc.vector.reciprocal(rbeta[:], sumexp[:])
                st["s_ps"], st["rbeta"] = s_ps, rbeta

            def tail(t, st):
                """s normalization, sT, out_pre, softmax2+mask, store."""
                r0 = t * P
                s_ps, rbeta = st["s_ps"], st["rbeta"]
                xt_sb = st["xt"]

                s_sb = sbp.tile([P, FIN], BF16, tag="s", name=f"s{t}")
                nc.scalar.mul(s_sb[:], s_ps[:], rbeta[:])

                sT = sbp.tile([P, KF, P], BF16, tag="sT", name=f"sT{t}")
                for kf in range(KF):
                    tp = pst_s.tile([P, P], BF16, tag="pst_s")
                    nc.tensor.transpose(
                        tp[:], s_sb[:, kf * P:(kf + 1) * P], identb[:]
                    )
                    nc.scalar.copy(sT[:, kf, :], tp[:])

                o_ps = pso.tile([P, D2], F32, tag="pso")
                nc.tensor.matmul(
                    o_ps[:], ones_sb[:], cvec_sb[:], start=True, stop=False,
                )
                for kd in range(KD):
                    nc.tensor.matmul(
                        o_ps[:], xt_sb[:, kd, :], w2_sb[:, kd, :],
                        start=False, stop=False,
                    )
                for kf in range(KF):
                    nc.tensor.matmul(
                        o_ps[:], sT[:, kf, :], w12_sb[:, kf, :],
                        start=False, stop=(kf == KF - 1),
                    )

                # softmax2: out1 = exp(o) / sumexp(o) (ACT exp w/ accum,
                # DVE reciprocal, ACT per-partition mul -- no Ln table).
                sumexp2 = smp.tile([P, 1], F32, tag="sumexp2")
                exp2 = sbp.tile([P, D2], F32, tag="exp2", name=f"e2{t}")
                nc.scalar.activation(
                    exp2[:], o_ps[:], AF.Exp,
                    bias=0.0, scale=1.0, accum_out=sumexp2[:],
                )
                rb2 = smp.tile([P, 1], F32, tag="rb2")
                nc.vector.reciprocal(rb2[:], sumexp2[:])
                pre = sbp.tile([P, D2], F32, tag="pre", name=f"pre{t}")
                nc.scalar.mul(pre[:], exp2[:], rb2[:])

                # logits = pre + NEG*(1-mask): add NEG everywhere, then
                # copy back `pre` where mask is nonzero (both on DVE).
                logits = sbp.tile([P, D2], F32, tag="logits", name=f"lg{t}")
                nc.vector.tensor_scalar(
                    out=logits[:], in0=pre[:], scalar1=NEG, scalar2=None,
                    op0=OP.add,
                )
                nc.vector.copy_predicated(logits[:], st["mask"][:], pre[:])
                nc.sync.dma_start(out_d[r0:r0 + P, :], logits[:])

            # pipeline: prologues run TWO tiles ahead so q(t+1) is ready
            # the moment tile t's dots finish, and tail(t) is emitted AFTER
            # alpha(t+1) -- the DVE/ACT queues are strict FIFO, so an
            # eagerly-emitted tail head-of-line-blocks the next tile's dot
            # products while the tail's PE/ACT chain resolves.  Deferring it
            # one tile gives the chain a full tile span to complete.
            states = {0: prologue(0)}
            states[1] = prologue(1)
            for t in range(rt):
                alpha_softmax(t, states[t])
                if t == 0:
                    build_w12_cvec()
                if t + 2 < rt:
                    states[t + 2] = prologue(t + 2)
                if t >= 1:
                    tail(t - 1, states[t - 1])
                    del states[t - 1]
            tail(rt - 1, states[rt - 1])

    nc.finalize()
    return nc


_NC_CACHE = {}


def _get_nc(bc):
    if bc not in _NC_CACHE:
        _NC_CACHE[bc] = build_nc(bc)
    return _NC_CACHE[bc]


def kernel(obs_x, others, action_mask, W1, b1, W2, b2, W3, b3, W4, b4,
           trace=False):
    obs_x = np.ascontiguousarray(np.asarray(obs_x, dtype=np.float32))
    others = np.ascontiguousarray(np.asarray(others, dtype=np.float32))
    action_mask = np.ascontiguousarray(np.asarray(action_mask, dtype=np.int32))
    W1 = np.ascontiguousarray(np.asarray(W1, dtype=np.float32))
    b1 = np.ascontiguousarray(np.asarray(b1, dtype=np.float32))
    W2 = np.ascontiguousarray(np.asarray(W2, dtype=np.float32))
    b2 = np.ascontiguousarray(np.asarray(b2, dtype=np.float32))

    bc = B // NCORES
    nc = _get_nc(bc)
    in_maps = []
    for c in range(NCORES):
        sl = slice(c * bc, (c + 1) * bc)
        in_maps.append({
            "obs_x": obs_x[sl],
            "others": others[sl],
            "action_mask": action_mask[sl],
            "W1": W1, "b1": b1, "W2": W2, "b2": b2,
        })
    res = run_bass_kernel_spmd(nc, in_maps, list(range(NCORES)), trace=trace)
    out = np.concatenate([res.results[c]["out"] for c in range(NCORES)], axis=0)
    if trace:
        return out, res
    return out
